# revision 6
# baseline (speedup 1.0000x reference)
"""NeuS volume-rendering kernel for 8 Trainium2 NeuronCores.

Math: with sig = sigmoid(s*sdf), the NeuS cumprod telescopes:
  1 - alpha[k] = sig[k+1]/sig[k]  =>  trans[i] = sig[i]/sig[0]
  weight[i] = relu(sig[i] - sig[i+1]) / sig[0]   (i = 1..S-2; w[0] = w[S-1] = 0)
  pixel[c]  = sum_s w*color_c + (1 - sum_s w)*bg_c
  invdepth  = sum_s w / z

Sharded data-parallel over rays across 8 cores; everything per-ray is local.
"""

import threading

import numpy as np

R_TOTAL = 65536
S = 128
N_CORES = 8
R_CORE = R_TOTAL // N_CORES  # 8192
P = 128  # rays per tile (partition dim)
TPB = 4  # tiles per super-tile


def _build(r_core: int):
    from contextlib import ExitStack

    import concourse.bacc as bacc
    import concourse.mybir as mybir
    import concourse.tile as tile

    f32 = mybir.dt.float32
    AF = mybir.ActivationFunctionType
    ALU = mybir.AluOpType

    T = r_core // P  # tiles per core
    NST = T // TPB  # super-tiles per core

    nc = bacc.Bacc("TRN2", target_bir_lowering=False, debug=False)

    sdf_d = nc.dram_tensor("sdf", [r_core, S], f32, kind="ExternalInput").ap()
    col_d = nc.dram_tensor("color", [r_core, S * 3], f32, kind="ExternalInput").ap()
    z_d = nc.dram_tensor("z_vals", [r_core, S], f32, kind="ExternalInput").ap()
    s_d = nc.dram_tensor("s", [1], f32, kind="ExternalInput").ap()
    bg_d = nc.dram_tensor("bg_color", [3], f32, kind="ExternalInput").ap()

    w_d = nc.dram_tensor("weight", [r_core, S], f32, kind="ExternalOutput").ap()
    pix_d = nc.dram_tensor("pixel", [r_core, 3], f32, kind="ExternalOutput").ap()
    inv_d = nc.dram_tensor("invdepth", [r_core], f32, kind="ExternalOutput").ap()

    # super-tile views: ray = (st*TPB + t)*P + p
    sdf_v = sdf_d.rearrange("(n t p) s -> n p t s", t=TPB, p=P)
    z_v = z_d.rearrange("(n t p) s -> n p t s", t=TPB, p=P)
    col_v = col_d.rearrange("(n t p) c -> n p t c", t=TPB, p=P)
    w_v = w_d.rearrange("(n t p) s -> n p t s", t=TPB, p=P)
    pix_v = pix_d.rearrange("(t p) c -> p t c", p=P)  # [128, T, 3]
    inv_v = inv_d.rearrange("(t p) -> p t", p=P)  # [128, T]
    sdf0_v = sdf_d.rearrange("(t p) s -> p t s", p=P)[:, :, 0:1]  # [128, T, 1]

    with tile.TileContext(nc) as tc, ExitStack() as ctx:
        const_pool = ctx.enter_context(tc.tile_pool(name="const", bufs=1))
        acc_pool = ctx.enter_context(tc.tile_pool(name="acc", bufs=1))
        in_pool = ctx.enter_context(tc.tile_pool(name="in", bufs=3))
        mid_pool = ctx.enter_context(tc.tile_pool(name="mid", bufs=3))
        scrv_pool = ctx.enter_context(tc.tile_pool(name="scrv", bufs=2))
        scrg_pool = ctx.enter_context(tc.tile_pool(name="scrg", bufs=2))

        # ---- constants: s, bg broadcast to all partitions (step-0 DMA) ----
        s_bc = const_pool.tile([P, 1], f32, tag="sbc")
        nc.sync.dma_start(s_bc[:], s_d[None, :].partition_broadcast(P))
        neg_s = const_pool.tile([P, 1], f32, tag="negs")
        nc.gpsimd.tensor_scalar_mul(neg_s[:], s_bc[:], -1.0)

        bg_bc = const_pool.tile([P, 3], f32, tag="bgbc")
        nc.sync.dma_start(bg_bc[:], bg_d[None, :].partition_broadcast(P))

        # ---- persistent accumulators ----
        pix_acc = acc_pool.tile([P, T, 3], f32, tag="pixacc")
        inv_acc = acc_pool.tile([P, T], f32, tag="invacc")
        wsum_acc = acc_pool.tile([P, T], f32, tag="wsumacc")

        # ---- rs prelude: rs = 1 + exp(-s * sdf[:, 0])  (uses exp table once,
        # before the main loop switches ACT to the sigmoid/relu set) ----
        sdf0_sb = const_pool.tile([P, T, 1], f32, tag="sdf0")
        nc.sync.dma_start(sdf0_sb[:], sdf0_v)
        e0 = const_pool.tile([P, T], f32, tag="e0")
        nc.scalar.activation(
            e0[:], sdf0_sb[:].rearrange("p t o -> p (t o)"), AF.Exp, scale=neg_s[:]
        )
        rs_all = const_pool.tile([P, T], f32, tag="rs")
        nc.gpsimd.tensor_scalar_add(rs_all[:], e0[:], 1.0)

        # ---- main loop over super-tiles ----
        for st in range(NST):
            sdf_t = in_pool.tile([P, TPB, S], f32, tag="sdf")
            nc.sync.dma_start(sdf_t[:], sdf_v[st])
            z_t = in_pool.tile([P, TPB, S], f32, tag="z")
            nc.sync.dma_start(z_t[:], z_v[st])
            col_t = in_pool.tile([P, TPB, S * 3], f32, tag="col")
            nc.sync.dma_start(col_t[:], col_v[st])

            sig_t = mid_pool.tile([P, TPB, S], f32, tag="sig")
            nc.scalar.activation(
                sig_t[:].rearrange("p t s -> p (t s)"),
                sdf_t[:].rearrange("p t s -> p (t s)"),
                AF.Sigmoid,
                scale=s_bc[:],
            )

            rz_t = mid_pool.tile([P, TPB, S], f32, tag="rz")
            nc.vector.reciprocal_approx_fast(
                rz_t[:].rearrange("p t s -> p (t s)"),
                z_t[:].rearrange("p t s -> p (t s)"),
            )

            # d[i] = sig[i] - sig[i+1] for i=1..126  (on GPSIMD)
            d_t = mid_pool.tile([P, TPB, S - 2], f32, tag="d")
            nc.gpsimd.tensor_sub(d_t[:], sig_t[:, :, 1 : S - 1], sig_t[:, :, 2:S])

            w_t = mid_pool.tile([P, TPB, S], f32, tag="w")
            # zero the edge columns (never written by the relu below)
            nc.gpsimd.memset(w_t[:, :, 0:1], 0.0)
            nc.gpsimd.memset(w_t[:, :, S - 1 : S], 0.0)

            for i in range(TPB):
                t = st * TPB + i
                # w = relu(d * rs); wsum accumulated by ACT
                nc.scalar.activation(
                    w_t[:, i, 1 : S - 1],
                    d_t[:, i, :],
                    AF.Relu,
                    scale=rs_all[:, t : t + 1],
                    accum_out=wsum_acc[:, t : t + 1],
                )
                # pixel channels: accum(color_c * w) on DVE
                for c in range(3):
                    scr = scrv_pool.tile([P, S], f32, tag="scr")
                    nc.vector.scalar_tensor_tensor(
                        out=scr[:],
                        in0=col_t[:, i, c :: 3],
                        scalar=1.0,
                        in1=w_t[:, i, :],
                        op0=ALU.mult,
                        op1=ALU.mult,
                        accum_out=pix_acc[:, t, c : c + 1],
                    )
                # invdepth: accum(rz * w) on DVE
                scr2 = scrv_pool.tile([P, S], f32, tag="scr")
                nc.vector.scalar_tensor_tensor(
                    out=scr2[:],
                    in0=rz_t[:, i, :],
                    scalar=1.0,
                    in1=w_t[:, i, :],
                    op0=ALU.mult,
                    op1=ALU.mult,
                    accum_out=inv_acc[:, t : t + 1],
                )

            nc.sync.dma_start(w_v[st], w_t[:])

        # ---- composite background: pixel += (1 - wsum) * bg ----
        w1 = const_pool.tile([P, T], f32, tag="w1")
        nc.vector.tensor_scalar(
            out=w1[:], in0=wsum_acc[:], scalar1=-1.0, scalar2=1.0,
            op0=ALU.mult, op1=ALU.add,
        )
        pix_fin = const_pool.tile([P, T, 3], f32, tag="pixfin")
        for c in range(3):
            nc.vector.scalar_tensor_tensor(
                out=pix_fin[:, :, c],
                in0=w1[:],
                scalar=bg_bc[:, c : c + 1],
                in1=pix_acc[:, :, c],
                op0=ALU.mult,
                op1=ALU.add,
            )

        nc.sync.dma_start(pix_v, pix_fin[:])
        nc.sync.dma_start(inv_v, inv_acc[:])

    nc.compile()
    return nc


_lock = threading.Lock()
_cache: dict = {}


def _get_nc(r_core: int):
    with _lock:
        if r_core not in _cache:
            _cache[r_core] = _build(r_core)
        return _cache[r_core]


def kernel(sdf, color, z_vals, s, bg_color):
    from concourse.bass_utils import run_bass_kernel_spmd

    sdf = np.ascontiguousarray(sdf, dtype=np.float32)
    color = np.ascontiguousarray(color, dtype=np.float32)
    z_vals = np.ascontiguousarray(z_vals, dtype=np.float32)
    s = np.ascontiguousarray(s, dtype=np.float32)
    bg_color = np.ascontiguousarray(bg_color, dtype=np.float32)

    nc = _get_nc(R_CORE)

    in_maps = []
    for k in range(N_CORES):
        lo, hi = k * R_CORE, (k + 1) * R_CORE
        in_maps.append(
            {
                "sdf": sdf[lo:hi],
                "color": color[lo:hi].reshape(R_CORE, S * 3),
                "z_vals": z_vals[lo:hi],
                "s": s,
                "bg_color": bg_color,
            }
        )

    res = run_bass_kernel_spmd(nc, in_maps, core_ids=list(range(N_CORES)))
    outs = res.results

    pixel = np.concatenate([r["pixel"] for r in outs], axis=0)
    invdepth = np.concatenate([r["invdepth"] for r in outs], axis=0)
    weight = np.concatenate([r["weight"] for r in outs], axis=0)
    return pixel, invdepth, weight
